# revision 29
# baseline (speedup 1.0000x reference)
"""Local (sliding-window) attention kernel for Trainium2, 8 NeuronCores.

Problem: B=4, T=2048, C=1024, window=16 (17 keys per query).
    q = x@Wq.T+bq; k = x@Wk.T+bk; v = x@Wv.T+bv
    scores = (q . k_win) / sqrt(C), softmax over the +-8 window, ctx = attn . v_win
    y = ctx@Wo.T + bo

Sharding: core i handles batch b = i//2, tokens [t0, t0+1024) with t0 = (i%2)*1024,
with an 8-token halo on each side for k/v (host-sliced, zero-padded at sequence
edges; validity handled by additive masks computed on host).

Device layout (per core, local token axis tl in [0, 1040) == global t0-8+tl):
    xT  [c, tl]    fp16  (host pre-transposed, zero-padded)
    qT  [co, 1024] fp16  = (x@Wq.T+bq)/sqrt(C), queries tl in [8, 1032)
    kT  [co, 1040] fp16  (tail [1024,1040) computed with 16-wide moving dim)
    v   [tl, co]   fp16  (8 full 128-token chunks + 16-token tail chunk, the
                          tail computed transposed (16-wide moving) then
                          PE-transposed into natural layout)
    per 128-query block b: keys are tl in [b*128, b*128+144); scores [128, 144]
    fp32 in PSUM + additive mask, exact softmax, P -> PE-transpose (128+16 cols)
    -> PV matmuls (K=128 chunk + K=16 tail) -> ctxT [c, 128] -> y = ctxT.T@WoT+bo
    emitted as fp16 (host converts to fp32).

All matmuls run in fp16 (1 row/cycle on PE); accumulation is fp32 in PSUM;
softmax is fp32.  Input DMAs are split into ~0.07-0.13MB pieces with >=512B
contiguous runs and issued from both HWDGE engines (Sync + Scalar) in
first-needed-first order: DMA descriptor issue costs ~0.7us per instruction,
which was the startup bottleneck.
"""

import numpy as np

B, T, C = 4, 2048, 1024
P = 128
CC = C // P            # 8 channel chunks
TQ = 1024              # queries per core
TK = 1040              # local kv token span (8-token halo each side)
NB = TQ // P           # 8 query blocks
WJ = 144               # key-window columns per block (128 + window)
HALF = 8               # window // 2
SCALE = 1.0 / 32.0     # 1/sqrt(C)
N_CORES = 8

_PROGRAM = None        # cached (nc, meta)
LAST_EXEC_NS = None
TRACE = False


def _apply_tile_drain_patch():
    """walrus (CoreV3) rejects the Tile tail-drain when it carries more than a
    couple of semaphore waits ("Too many sync wait commands").  Split the waits:
    keep one on the drain, emit the rest as single-wait SP instructions."""
    import bass_rust
    import concourse.tile as tile
    from concourse.vector_clock import ScopedClock

    if getattr(tile.TileContext, "_drain_split_patch", False):
        return

    def _drain_and_barrier(self, tick_clock, wait_clock):
        nc = self.nc
        drain_inst = nc.sync.drain()
        wait_clock.add_sem_waits(
            drain_inst.ins, ScopedClock({None: tick_clock.global_clock})
        )
        si = drain_inst.ins.sync_info
        waits = list(si.on_wait)
        if len(waits) > 1:
            byid = {h.num: h for h in self.sems.allocated().values()}
            drain_inst.ins.sync_info = bass_rust.SyncInfo(
                on_wait=waits[:1], on_update=list(si.on_update)
            )
            for w in waits[1:]:
                nc.sync.wait_ge(byid[w.id], w.wait_value)

        # Single final barrier; skip the per-range semaphore clear + second
        # barrier: the NEFF epilogue already zeroes every semaphore (observed
        # in the trace as per-engine $S[n]=0 chains), so the Tile-level clear
        # only lengthened the in-kernel tail.
        nc.all_engine_barrier()
        assert self.sems is not None
        popped = nc._tile_sem_poison_stack.pop()
        assert popped is self._sem_poison
        nc._state.prepend_free_semaphores(
            [s.num for s in self.sems.allocated().values()]
        )

    tile.TileContext._drain_and_barrier = _drain_and_barrier
    tile.TileContext._drain_split_patch = True


def _split_excess_waits(nc, limit=1):
    """This walrus build rejects instructions carrying more than a couple of
    embedded semaphore waits ("Too many sync wait commands").  Hoist excess
    waits into same-engine NoOp instructions placed immediately before."""
    import bass_rust
    import concourse.mybir as mybir

    cnt = 0
    for f in nc.m.functions:
        for bb in f.blocks:
            changed = False
            out = []
            for inst in bb.instructions:
                si = inst.sync_info
                if si is None:
                    out.append(inst)
                    continue
                waits = list(si.on_wait)
                if len(waits) > limit:
                    changed = True
                    extra, keep = waits[:-limit], waits[-limit:]
                    for i in range(0, len(extra), limit):
                        nop = mybir.InstNoOp(name=f"waitsplit_{cnt}", ins=[], outs=[])
                        cnt += 1
                        nop.engine = inst.engine
                        nop.sync_info = bass_rust.SyncInfo(
                            on_wait=extra[i: i + limit], on_update=[]
                        )
                        out.append(nop)
                    inst.sync_info = bass_rust.SyncInfo(
                        on_wait=keep, on_update=list(si.on_update)
                    )
                out.append(inst)
            if changed:
                bb.instructions = out
    return cnt


def _build_program():
    import concourse.bass as bass
    import concourse.mybir as mybir
    import concourse.tile as tile
    from concourse.masks import make_identity

    _apply_tile_drain_patch()

    dt = mybir.dt
    f16 = dt.float16
    f32 = dt.float32
    AF = mybir.ActivationFunctionType
    AX = mybir.AxisListType

    nc = bass.Bass("TRN2", target_bir_lowering=False, debug=False)

    xT_d = nc.dram_tensor("xT", [C, TK], f16, kind="ExternalInput").ap()
    wq_d = nc.dram_tensor("wqT", [C, C], f16, kind="ExternalInput").ap()
    wk_d = nc.dram_tensor("wkT", [C, C], f16, kind="ExternalInput").ap()
    wv_d = nc.dram_tensor("wvT", [C, C], f16, kind="ExternalInput").ap()
    wo_d = nc.dram_tensor("woT", [C, C], f16, kind="ExternalInput").ap()
    bqs_d = nc.dram_tensor("bqs", [P, CC], f32, kind="ExternalInput").ap()
    bkt_d = nc.dram_tensor("bkt", [P, CC], f32, kind="ExternalInput").ap()
    bvt_d = nc.dram_tensor("bvt", [P, CC], f32, kind="ExternalInput").ap()
    bob_d = nc.dram_tensor("bob", [P, C], f32, kind="ExternalInput").ap()
    mask_d = nc.dram_tensor("mask", [NB, P, WJ], f32, kind="ExternalInput").ap()
    y_d = nc.dram_tensor("y", [TQ, C], f16, kind="ExternalOutput").ap()

    with tile.TileContext(nc) as tc:
        from contextlib import ExitStack

        with ExitStack() as ctx:
            consts = ctx.enter_context(tc.tile_pool(name="consts", bufs=1))
            qkv = ctx.enter_context(tc.tile_pool(name="qkv", bufs=1))
            work = ctx.enter_context(tc.tile_pool(name="work", bufs=3))
            ctxp = ctx.enter_context(tc.tile_pool(name="ctxp", bufs=2))
            ptp = ctx.enter_context(tc.tile_pool(name="ptp", bufs=4))
            yp = ctx.enter_context(tc.tile_pool(name="yp", bufs=3))
            ps_big = ctx.enter_context(tc.tile_pool(name="ps_big", bufs=4, space="PSUM"))
            ps_s = ctx.enter_context(tc.tile_pool(name="ps_s", bufs=2, space="PSUM"))
            ps_sm = ctx.enter_context(tc.tile_pool(name="ps_sm", bufs=2, space="PSUM"))

            # ---- persistent SBUF tensors ----
            wq_sb = consts.tile([P, CC, C], f16, tag="wq")
            wk_sb = consts.tile([P, CC, C], f16, tag="wk")
            wv_sb = consts.tile([P, CC, C], f16, tag="wv")
            wo_sb = consts.tile([P, CC, C], f16, tag="wo")
            xT_sb = consts.tile([P, CC, TK], f16, tag="xT")
            bq_sb = consts.tile([P, CC], f32, tag="bq")
            bk_sb = consts.tile([P, CC], f32, tag="bk")
            bv_sb = consts.tile([P, CC], f32, tag="bv")
            bo_sb = consts.tile([P, C], f32, tag="bo")
            mask_sb = consts.tile([P, NB, WJ], f32, tag="mask")
            ident = consts.tile([P, P], f16, tag="ident")
            vt16 = consts.tile([P, CC, 16], f16, tag="vt16")

            qT_sb = qkv.tile([P, CC, TQ], f16, tag="qT")
            kT_sb = qkv.tile([P, CC, TK], f16, tag="kT")
            v_sb = qkv.tile([P, TK // P + 1, C], f16, tag="v")

            # PE warmup on a scratch tile: fills the initial DMA wait with
            # discarded matmuls so HAM un-throttles before the real work.
            scratch = consts.tile([P, 512], f16, tag="scratch")
            nc.gpsimd.memset(scratch[:], 0.0)
            ps_w = ps_big.tile([P, 512], f32, tag="big", name="ps_warm")
            NWARM = 14
            for i in range(NWARM):
                nc.tensor.matmul(
                    ps_w,
                    lhsT=scratch[:, 0:128],
                    rhs=scratch[:],
                    start=(i == 0),
                    stop=(i == NWARM - 1),
                )

            # ---- input DMAs ----
            # Descriptor issue costs ~0.7us each, so the issue ORDER and the
            # engine split (sync + scalar both have hardware DGE) determine
            # when the first projection can start.  Pieces are sized so each
            # queue transfer is ~3-11us and contiguous runs are >=512B.
            xT_r = xT_d.rearrange("(cc p) t -> cc p t", p=P)
            wq_r = wq_d.rearrange("(cc p) co -> p cc co", p=P)
            wk_r = wk_d.rearrange("(cc p) co -> p cc co", p=P)
            wv_r = wv_d.rearrange("(cc p) co -> p cc co", p=P)
            wo_r = wo_d.rearrange("(cc p) co -> p cc co", p=P)
            mask_r = mask_d.rearrange("b p j -> p b j")

            def dma_x(eng, cc, a, b):
                eng.dma_start(xT_sb[:, cc, a:b], xT_r[cc][:, a:b])

            def dma_w(eng, sb, r, ci0, ci1, a, b):
                eng.dma_start(sb[:, ci0:ci1, a:b], r[:, ci0:ci1, a:b])

            # The first q superblock is 256 tokens wide ([8:264)) so the gate
            # is xA (cols 0:288) + wq co[0:256] (17 pieces); the rest of wq
            # follows in consumption order (co-slices), then xB/xC, then the
            # k/v/o weights which are needed much later.
            sca = nc.scalar
            syn = nc.sync
            for cc in (1, 3, 5, 7):
                dma_x(sca, cc, 0, 288)
            for ci in (4, 5, 6, 7):
                dma_w(sca, wq_sb, wq_r, ci, ci + 1, 0, 256)
            for j in (2, 3):
                dma_w(sca, wq_sb, wq_r, 2 * j, 2 * j + 2, 256, 512)
            for j in (2, 3):
                dma_w(sca, wq_sb, wq_r, 2 * j, 2 * j + 2, 512, 768)
            for j in (2, 3):
                dma_w(sca, wq_sb, wq_r, 2 * j, 2 * j + 2, 768, 1024)
            for cc in (5, 7):
                dma_x(sca, cc, 288, 576)

            syn.dma_start(bq_sb[:], bqs_d[:])
            for cc in (0, 2, 4, 6):
                dma_x(syn, cc, 0, 288)
            for ci in (0, 1, 2, 3):
                dma_w(syn, wq_sb, wq_r, ci, ci + 1, 0, 256)
            for j in (0, 1):
                dma_w(syn, wq_sb, wq_r, 2 * j, 2 * j + 2, 256, 512)
            for j in (0, 1):
                dma_w(syn, wq_sb, wq_r, 2 * j, 2 * j + 2, 512, 768)
            for j in (0, 1):
                dma_w(syn, wq_sb, wq_r, 2 * j, 2 * j + 2, 768, 1024)
            for cc in (0, 1, 2, 3, 4, 6):
                dma_x(syn, cc, 288, 576)
            for cc in range(CC):
                dma_x(syn, cc, 576, TK)
            syn.dma_start(bk_sb[:], bkt_d[:])
            for h in range(2):
                for j in range(4):
                    dma_w(syn, wk_sb, wk_r, 2 * j, 2 * j + 2, h * 512, (h + 1) * 512)
            syn.dma_start(bv_sb[:], bvt_d[:])
            for h in range(2):
                for j in range(4):
                    dma_w(syn, wv_sb, wv_r, 2 * j, 2 * j + 2, h * 512, (h + 1) * 512)
            syn.dma_start(mask_sb[:, 0:4, :], mask_r[:, 0:4, :])
            syn.dma_start(mask_sb[:, 4:8, :], mask_r[:, 4:8, :])
            syn.dma_start(bo_sb[:], bob_d[:])
            for h in range(2):
                for j in range(4):
                    dma_w(syn, wo_sb, wo_r, 2 * j, 2 * j + 2, h * 512, (h + 1) * 512)

            make_identity(nc, ident[:])

            # ---- projections ----
            # qT[co, t] for the 1024 queries (tl offset 8); superblocks of
            # (256, 256, 512) tokens so the first groups start on minimal DMA
            for q0, wdt in ((0, 256), (256, 256), (512, 512)):
                for cc in range(CC):
                    ps_full = ps_big.tile([P, 512], f32, tag="big")
                    ps = ps_full[:, :wdt]
                    for ci in range(CC):
                        nc.tensor.matmul(
                            ps,
                            lhsT=wq_sb[:, ci, cc * P:(cc + 1) * P],
                            rhs=xT_sb[:, ci, HALF + q0: HALF + q0 + wdt],
                            start=(ci == 0),
                            stop=(ci == CC - 1),
                        )
                    if q0 == 0 and cc < 2:
                        # first two evictions on Vector (idle then): the
                        # Scalar engine is still issuing its DMA batch and a
                        # late first eviction stalls the PE on PSUM slots
                        nc.vector.tensor_scalar(
                            qT_sb[:, cc, q0: q0 + wdt],
                            ps,
                            SCALE,
                            bq_sb[:, cc: cc + 1],
                            mybir.AluOpType.mult,
                            mybir.AluOpType.add,
                        )
                    else:
                        nc.scalar.activation(
                            qT_sb[:, cc, q0: q0 + wdt],
                            ps,
                            AF.Identity,
                            bias=bq_sb[:, cc: cc + 1],
                            scale=SCALE,
                        )
            # kT[co, tl] over the padded 1040 kv range
            for off, wdt in ((0, 512), (512, 512), (1024, 16)):
                for cc in range(CC):
                    if wdt == 512:
                        ps = ps_big.tile([P, 512], f32, tag="big", name="ps_k")
                    else:
                        ps_full = ps_sm.tile([P, P], f32, tag="sm", name="ps_kt")
                        ps = ps_full[:, :wdt]
                    for ci in range(CC):
                        nc.tensor.matmul(
                            ps,
                            lhsT=wk_sb[:, ci, cc * P:(cc + 1) * P],
                            rhs=xT_sb[:, ci, off: off + wdt],
                            start=(ci == 0),
                            stop=(ci == CC - 1),
                        )
                    nc.scalar.activation(
                        kT_sb[:, cc, off: off + wdt],
                        ps,
                        AF.Identity,
                        bias=bk_sb[:, cc: cc + 1],
                    )
            # v[tl, co] natural layout, 8 full chunks (bv is folded into ctxT
            # later)
            for ch in range(8):
                for h in range(2):
                    ps = ps_big.tile([P, 512], f32, tag="big")
                    for ci in range(CC):
                        nc.tensor.matmul(
                            ps,
                            lhsT=xT_sb[:, ci, ch * P:(ch + 1) * P],
                            rhs=wv_sb[:, ci, h * 512:(h + 1) * 512],
                            start=(ci == 0),
                            stop=(ci == CC - 1),
                        )
                    nc.vector.tensor_copy(v_sb[:, ch, h * 512:(h + 1) * 512], ps)
            # v tail (tl 1024:1040): computed transposed with a 16-wide moving
            # dim (1k rows instead of 8k), then PE-transposed into natural
            # layout.
            for cc in range(CC):
                ps_full = ps_sm.tile([P, P], f32, tag="sm", name="ps_vt")
                ps = ps_full[:, :16]
                for ci in range(CC):
                    nc.tensor.matmul(
                        ps,
                        lhsT=wv_sb[:, ci, cc * P:(cc + 1) * P],
                        rhs=xT_sb[:, ci, 1024:TK],
                        start=(ci == 0),
                        stop=(ci == CC - 1),
                    )
                nc.vector.tensor_copy(vt16[:, cc, :], ps)
            for cc in range(CC):
                pst = ps_sm.tile([P, P], f16, tag="sm", name="ps_vtt")
                nc.tensor.transpose(pst[0:16, :], vt16[:, cc, :], ident[:])
                nc.vector.tensor_copy(
                    v_sb[0:16, 8, cc * P:(cc + 1) * P], pst[0:16, :]
                )

            # ---- attention + output projection, per 128-query block ----
            # The output projection of block b is emitted between block b+1's
            # P-transposes and its PV matmuls (software pipelining): the PE
            # then has ~4us of o-proj work to hide the PSUM->SBUF copy
            # latency of the transposed P that gates PV.
            def oproj(b, ctx_blk, nh=2):
                # nh=4 (256-wide pieces) for the final block: the last y DMA
                # is then half as large, shortening the kernel tail.
                y_sb = yp.tile([P, C], f16, tag="y")
                w = C // nh
                for h in range(nh):
                    psy_full = ps_big.tile([P, 512], f32, tag="big")
                    psy = psy_full[:, :w]
                    for ci in range(CC):
                        nc.tensor.matmul(
                            psy,
                            lhsT=ctx_blk[:, ci, :],
                            rhs=wo_sb[:, ci, h * w:(h + 1) * w],
                            start=(ci == 0),
                            stop=(ci == CC - 1),
                        )
                    nc.vector.tensor_add(
                        y_sb[:, h * w:(h + 1) * w], psy, bo_sb[:, h * w:(h + 1) * w]
                    )
                    nc.sync.dma_start(
                        y_d[b * P:(b + 1) * P, h * w:(h + 1) * w],
                        y_sb[:, h * w:(h + 1) * w],
                    )

            prev = None
            for b in range(NB):
                ps = ps_s.tile([P, WJ], f32, tag="s")
                for cc in range(CC):
                    nc.tensor.matmul(
                        ps,
                        lhsT=qT_sb[:, cc, b * P:(b + 1) * P],
                        rhs=kT_sb[:, cc, b * P: b * P + WJ],
                        start=(cc == 0),
                        stop=(cc == CC - 1),
                    )
                S = work.tile([P, WJ], f32, tag="S")
                nc.vector.tensor_add(S, ps, mask_sb[:, b, :])
                negm = work.tile([P, 1], f32, tag="negm")
                nc.vector.reduce_max(negm, S, axis=AX.X, negate=True)
                P32 = work.tile([P, WJ], f32, tag="P32")
                ssum = work.tile([P, 1], f32, tag="ssum")
                nc.scalar.activation(
                    P32, S, AF.Exp, bias=negm[:, 0:1], accum_out=ssum[:, 0:1]
                )
                rr = work.tile([P, 1], f32, tag="rr")
                nc.vector.reciprocal(rr, ssum)
                P16 = work.tile([P, WJ], f16, tag="P16")
                nc.vector.tensor_scalar_mul(P16, P32, rr[:, 0:1])

                # pt0 copy on Scalar, pt1 on Vector: the two PSUM->SBUF copies
                # of the transposed P run in parallel and neither sits behind
                # the previous block's y bias-adds in the Vector queue.
                pps0 = ps_sm.tile([P, P], f16, tag="sm", name="ps_pt0")
                nc.tensor.transpose(pps0, P16[:, 0:P], ident[:])
                pt0 = ptp.tile([P, P], f16, tag="ptt")
                nc.scalar.activation(pt0, pps0, AF.Copy)
                pps1 = ps_sm.tile([P, P], f16, tag="sm", name="ps_pt1")
                nc.tensor.transpose(pps1[0:16, :], P16[:, P:WJ], ident[:])
                pt1 = ptp.tile([P, P], f16, tag="ptt")
                nc.vector.tensor_copy(pt1[0:16, :], pps1[0:16, :])

                if prev is not None:
                    oproj(*prev)

                ctx_blk = ctxp.tile([P, CC, P], f16, tag="ctxT")
                for cs in range(CC):
                    pc_full = ps_sm.tile([P, P], f32, tag="sm", name="ps_ct")
                    pc = pc_full
                    nc.tensor.matmul(
                        pc,
                        lhsT=v_sb[:, b, cs * P:(cs + 1) * P],
                        rhs=pt0[:],
                        start=True,
                        stop=False,
                    )
                    nc.tensor.matmul(
                        pc,
                        lhsT=v_sb[0:16, b + 1, cs * P:(cs + 1) * P],
                        rhs=pt1[0:16, :],
                        start=False,
                        stop=True,
                    )
                    nc.scalar.activation(
                        ctx_blk[:, cs, :], pc, AF.Identity, bias=bv_sb[:, cs: cs + 1]
                    )

                prev = (b, ctx_blk)
            oproj(*prev, nh=4)

    _split_excess_waits(nc)
    return nc


def _host_inputs(x, Wq, bq, Wk, bk, Wv, bv, Wo, bo):
    """Build per-core input maps (shared weight arrays across cores)."""
    f16 = np.float16
    wqT = np.ascontiguousarray(np.asarray(Wq, np.float32).T).astype(f16)
    wkT = np.ascontiguousarray(np.asarray(Wk, np.float32).T).astype(f16)
    wvT = np.ascontiguousarray(np.asarray(Wv, np.float32).T).astype(f16)
    woT = np.ascontiguousarray(np.asarray(Wo, np.float32).T).astype(f16)

    def tile_bias(b_, scale=1.0):
        b_ = np.asarray(b_, np.float32) * scale
        return np.ascontiguousarray(b_.reshape(CC, P).T).astype(np.float32)

    bqs = tile_bias(bq, SCALE)
    bkt = tile_bias(bk)
    bvt = tile_bias(bv)
    bob = np.ascontiguousarray(
        np.broadcast_to(np.asarray(bo, np.float32), (P, C))
    ).astype(np.float32)

    x = np.asarray(x, np.float32)
    in_maps = []
    for core in range(N_CORES):
        bidx = core // 2
        t0 = (core % 2) * TQ
        lo = t0 - HALF
        xT = np.zeros((C, TK), f16)
        s0 = max(lo, 0)
        s1 = min(lo + TK, T)
        xT[:, s0 - lo: s1 - lo] = x[bidx, s0:s1, :].T.astype(f16)

        ii = np.arange(P)[None, :, None]
        jj = np.arange(WJ)[None, None, :]
        bb = np.arange(NB)[:, None, None]
        band = (jj - ii >= 0) & (jj - ii <= 2 * HALF)
        gk = lo + bb * P + jj
        valid = band & (gk >= 0) & (gk < T)
        mask = np.where(valid, np.float32(0.0), np.float32(-1e30))
        mask = np.ascontiguousarray(np.broadcast_to(mask, (NB, P, WJ)), np.float32)

        in_maps.append(
            {
                "xT": xT,
                "wqT": wqT,
                "wkT": wkT,
                "wvT": wvT,
                "woT": woT,
                "bqs": bqs,
                "bkt": bkt,
                "bvt": bvt,
                "bob": bob,
                "mask": mask,
            }
        )
    return in_maps


def kernel(x, Wq, bq, Wk, bk, Wv, bv, Wo, bo, window):
    global _PROGRAM, LAST_EXEC_NS
    assert int(window) == 2 * HALF

    from concourse import bass_utils

    if _PROGRAM is None:
        _PROGRAM = _build_program()
    nc = _PROGRAM

    in_maps = _host_inputs(x, Wq, bq, Wk, bk, Wv, bv, Wo, bo)
    res = bass_utils.run_bass_kernel_spmd(
        nc, in_maps, core_ids=list(range(N_CORES)), trace=TRACE
    )
    LAST_EXEC_NS = res.exec_time_ns

    out = np.empty((B, T, C), np.float32)
    for core in range(N_CORES):
        bidx = core // 2
        t0 = (core % 2) * TQ
        out[bidx, t0: t0 + TQ, :] = res.results[core]["y"].astype(np.float32)
    return out


# revision 31
# speedup vs baseline: 1.0211x; 1.0211x over previous
"""Local (sliding-window) attention kernel for Trainium2, 8 NeuronCores.

Problem: B=4, T=2048, C=1024, window=16 (17 keys per query).
    q = x@Wq.T+bq; k = x@Wk.T+bk; v = x@Wv.T+bv
    scores = (q . k_win) / sqrt(C), softmax over the +-8 window, ctx = attn . v_win
    y = ctx@Wo.T + bo

Sharding: core i handles batch b = i//2, tokens [t0, t0+1024) with t0 = (i%2)*1024,
with an 8-token halo on each side for k/v (host-sliced, zero-padded at sequence
edges; validity handled by additive masks computed on host).

Device layout (per core, local token axis tl in [0, 1040) == global t0-8+tl):
    xT  [c, tl]    fp16  (host pre-transposed, zero-padded)
    qT  [co, 1024] fp16  = (x@Wq.T+bq)/sqrt(C), queries tl in [8, 1032)
    kT  [co, 1040] fp16  (tail [1024,1040) computed with 16-wide moving dim)
    v   [tl, co]   fp16  (8 full 128-token chunks + 16-token tail chunk, the
                          tail computed transposed (16-wide moving) then
                          PE-transposed into natural layout)
    per 128-query block b: keys are tl in [b*128, b*128+144); scores [128, 144]
    fp32 in PSUM + additive mask, exact softmax, P -> PE-transpose (128+16 cols)
    -> PV matmuls (K=128 chunk + K=16 tail) -> ctxT [c, 128] -> y = ctxT.T@WoT+bo
    emitted as fp16 (host converts to fp32).

All matmuls run in fp16 (1 row/cycle on PE); accumulation is fp32 in PSUM;
softmax is fp32.  Input DMAs are split into ~0.07-0.13MB pieces with >=512B
contiguous runs and issued from both HWDGE engines (Sync + Scalar) in
first-needed-first order: DMA descriptor issue costs ~0.7us per instruction,
which was the startup bottleneck.
"""

import numpy as np

B, T, C = 4, 2048, 1024
P = 128
CC = C // P            # 8 channel chunks
TQ = 1024              # queries per core
TK = 1040              # local kv token span (8-token halo each side)
NB = TQ // P           # 8 query blocks
WJ = 144               # key-window columns per block (128 + window)
HALF = 8               # window // 2
SCALE = 1.0 / 32.0     # 1/sqrt(C)
N_CORES = 8

_PROGRAM = None        # cached (nc, meta)
LAST_EXEC_NS = None
TRACE = False


def _apply_tile_drain_patch():
    """walrus (CoreV3) rejects the Tile tail-drain when it carries more than a
    couple of semaphore waits ("Too many sync wait commands").  Split the waits:
    keep one on the drain, emit the rest as single-wait SP instructions."""
    import bass_rust
    import concourse.tile as tile
    from concourse.vector_clock import ScopedClock

    if getattr(tile.TileContext, "_drain_split_patch", False):
        return

    def _drain_and_barrier(self, tick_clock, wait_clock):
        nc = self.nc
        drain_inst = nc.sync.drain()
        wait_clock.add_sem_waits(
            drain_inst.ins, ScopedClock({None: tick_clock.global_clock})
        )
        si = drain_inst.ins.sync_info
        waits = list(si.on_wait)
        if len(waits) > 1:
            byid = {h.num: h for h in self.sems.allocated().values()}
            drain_inst.ins.sync_info = bass_rust.SyncInfo(
                on_wait=waits[:1], on_update=list(si.on_update)
            )
            for w in waits[1:]:
                nc.sync.wait_ge(byid[w.id], w.wait_value)

        # Single final barrier; skip the per-range semaphore clear + second
        # barrier: the NEFF epilogue already zeroes every semaphore (observed
        # in the trace as per-engine $S[n]=0 chains), so the Tile-level clear
        # only lengthened the in-kernel tail.
        nc.all_engine_barrier()
        assert self.sems is not None
        popped = nc._tile_sem_poison_stack.pop()
        assert popped is self._sem_poison
        nc._state.prepend_free_semaphores(
            [s.num for s in self.sems.allocated().values()]
        )

    tile.TileContext._drain_and_barrier = _drain_and_barrier
    tile.TileContext._drain_split_patch = True


def _split_excess_waits(nc, limit=1):
    """This walrus build rejects instructions carrying more than a couple of
    embedded semaphore waits ("Too many sync wait commands").  Hoist excess
    waits into same-engine NoOp instructions placed immediately before."""
    import bass_rust
    import concourse.mybir as mybir

    cnt = 0
    for f in nc.m.functions:
        for bb in f.blocks:
            changed = False
            out = []
            for inst in bb.instructions:
                si = inst.sync_info
                if si is None:
                    out.append(inst)
                    continue
                waits = list(si.on_wait)
                if len(waits) > limit:
                    changed = True
                    extra, keep = waits[:-limit], waits[-limit:]
                    for i in range(0, len(extra), limit):
                        nop = mybir.InstNoOp(name=f"waitsplit_{cnt}", ins=[], outs=[])
                        cnt += 1
                        nop.engine = inst.engine
                        nop.sync_info = bass_rust.SyncInfo(
                            on_wait=extra[i: i + limit], on_update=[]
                        )
                        out.append(nop)
                    inst.sync_info = bass_rust.SyncInfo(
                        on_wait=keep, on_update=list(si.on_update)
                    )
                out.append(inst)
            if changed:
                bb.instructions = out
    return cnt


def _build_program():
    import concourse.bass as bass
    import concourse.mybir as mybir
    import concourse.tile as tile
    from concourse.masks import make_identity

    _apply_tile_drain_patch()

    dt = mybir.dt
    f16 = dt.float16
    f32 = dt.float32
    AF = mybir.ActivationFunctionType
    AX = mybir.AxisListType

    nc = bass.Bass("TRN2", target_bir_lowering=False, debug=False)

    xT_d = nc.dram_tensor("xT", [C, TK], f16, kind="ExternalInput").ap()
    wq_d = nc.dram_tensor("wqT", [C, C], f16, kind="ExternalInput").ap()
    wk_d = nc.dram_tensor("wkT", [C, C], f16, kind="ExternalInput").ap()
    wv_d = nc.dram_tensor("wvT", [C, C], f16, kind="ExternalInput").ap()
    wo_d = nc.dram_tensor("woT", [C, C], f16, kind="ExternalInput").ap()
    bqs_d = nc.dram_tensor("bqs", [P, CC], f32, kind="ExternalInput").ap()
    bkt_d = nc.dram_tensor("bkt", [P, CC], f32, kind="ExternalInput").ap()
    bvt_d = nc.dram_tensor("bvt", [P, CC], f32, kind="ExternalInput").ap()
    bob_d = nc.dram_tensor("bob", [P, C], f32, kind="ExternalInput").ap()
    mask_d = nc.dram_tensor("mask", [NB, P, WJ], f32, kind="ExternalInput").ap()
    y_d = nc.dram_tensor("y", [TQ, C], f16, kind="ExternalOutput").ap()

    with tile.TileContext(nc) as tc:
        from contextlib import ExitStack

        with ExitStack() as ctx:
            consts = ctx.enter_context(tc.tile_pool(name="consts", bufs=1))
            qkv = ctx.enter_context(tc.tile_pool(name="qkv", bufs=1))
            work = ctx.enter_context(tc.tile_pool(name="work", bufs=3))
            ctxp = ctx.enter_context(tc.tile_pool(name="ctxp", bufs=2))
            ptp = ctx.enter_context(tc.tile_pool(name="ptp", bufs=4))
            yp = ctx.enter_context(tc.tile_pool(name="yp", bufs=3))
            ps_big = ctx.enter_context(tc.tile_pool(name="ps_big", bufs=4, space="PSUM"))
            ps_s = ctx.enter_context(tc.tile_pool(name="ps_s", bufs=2, space="PSUM"))
            ps_sm = ctx.enter_context(tc.tile_pool(name="ps_sm", bufs=2, space="PSUM"))

            # ---- persistent SBUF tensors ----
            wq_sb = consts.tile([P, CC, C], f16, tag="wq")
            wk_sb = consts.tile([P, CC, C], f16, tag="wk")
            wv_sb = consts.tile([P, CC, C], f16, tag="wv")
            wo_sb = consts.tile([P, CC, C], f16, tag="wo")
            xT_sb = consts.tile([P, CC, TK], f16, tag="xT")
            bq_sb = consts.tile([P, CC], f32, tag="bq")
            bk_sb = consts.tile([P, CC], f32, tag="bk")
            bv_sb = consts.tile([P, CC], f32, tag="bv")
            bo_sb = consts.tile([P, C], f32, tag="bo")
            mask_sb = consts.tile([P, NB, WJ], f32, tag="mask")
            ident = consts.tile([P, P], f16, tag="ident")
            vt16 = consts.tile([P, CC, 16], f16, tag="vt16")

            qT_sb = qkv.tile([P, CC, TQ], f16, tag="qT")
            kT_sb = qkv.tile([P, CC, TK], f16, tag="kT")
            v_sb = qkv.tile([P, TK // P + 1, C], f16, tag="v")

            # PE warmup on a scratch tile: fills the initial DMA wait with
            # discarded matmuls so HAM un-throttles before the real work.
            scratch = consts.tile([P, 512], f16, tag="scratch")
            nc.gpsimd.memset(scratch[:], 0.0)
            ps_w = ps_big.tile([P, 512], f32, tag="big", name="ps_warm")
            NWARM = 14
            for i in range(NWARM):
                nc.tensor.matmul(
                    ps_w,
                    lhsT=scratch[:, 0:128],
                    rhs=scratch[:],
                    start=(i == 0),
                    stop=(i == NWARM - 1),
                )

            # ---- input DMAs ----
            # Descriptor issue costs ~0.7us each, so the issue ORDER and the
            # engine split (sync + scalar both have hardware DGE) determine
            # when the first projection can start.  Pieces are sized so each
            # queue transfer is ~3-11us and contiguous runs are >=512B.
            xT_r = xT_d.rearrange("(cc p) t -> cc p t", p=P)
            wq_r = wq_d.rearrange("(cc p) co -> p cc co", p=P)
            wk_r = wk_d.rearrange("(cc p) co -> p cc co", p=P)
            wv_r = wv_d.rearrange("(cc p) co -> p cc co", p=P)
            wo_r = wo_d.rearrange("(cc p) co -> p cc co", p=P)
            mask_r = mask_d.rearrange("b p j -> p b j")

            def dma_x(eng, cc, a, b):
                eng.dma_start(xT_sb[:, cc, a:b], xT_r[cc][:, a:b])

            def dma_w(eng, sb, r, ci0, ci1, a, b):
                eng.dma_start(sb[:, ci0:ci1, a:b], r[:, ci0:ci1, a:b])

            # The first q superblock is 256 tokens wide ([8:264)) so the gate
            # is xA (cols 0:288) + wq co[0:256] (17 pieces); the rest of wq
            # follows in consumption order (co-slices), then xB/xC, then the
            # k/v/o weights which are needed much later.
            sca = nc.scalar
            syn = nc.sync
            for cc in (1, 3, 5, 7):
                dma_x(sca, cc, 0, 288)
            for ci in (4, 5, 6, 7):
                dma_w(sca, wq_sb, wq_r, ci, ci + 1, 0, 256)
            for j in (2, 3):
                dma_w(sca, wq_sb, wq_r, 2 * j, 2 * j + 2, 256, 512)
            for j in (2, 3):
                dma_w(sca, wq_sb, wq_r, 2 * j, 2 * j + 2, 512, 768)
            for j in (2, 3):
                dma_w(sca, wq_sb, wq_r, 2 * j, 2 * j + 2, 768, 1024)

            syn.dma_start(bq_sb[:], bqs_d[:])
            for cc in (0, 2, 4, 6):
                dma_x(syn, cc, 0, 288)
            for ci in (0, 1, 2, 3):
                dma_w(syn, wq_sb, wq_r, ci, ci + 1, 0, 256)
            for j in (0, 1):
                dma_w(syn, wq_sb, wq_r, 2 * j, 2 * j + 2, 256, 512)
            for j in (0, 1):
                dma_w(syn, wq_sb, wq_r, 2 * j, 2 * j + 2, 512, 768)
            for j in (0, 1):
                dma_w(syn, wq_sb, wq_r, 2 * j, 2 * j + 2, 768, 1024)
            for cc in range(CC):
                dma_x(syn, cc, 288, 576)
            for cc in range(CC):
                dma_x(syn, cc, 576, TK)
            syn.dma_start(bk_sb[:], bkt_d[:])
            for h in range(2):
                for j in range(4):
                    dma_w(syn, wk_sb, wk_r, 2 * j, 2 * j + 2, h * 512, (h + 1) * 512)
            syn.dma_start(bv_sb[:], bvt_d[:])
            for h in range(2):
                for j in range(4):
                    dma_w(syn, wv_sb, wv_r, 2 * j, 2 * j + 2, h * 512, (h + 1) * 512)
            syn.dma_start(mask_sb[:, 0:4, :], mask_r[:, 0:4, :])
            syn.dma_start(mask_sb[:, 4:8, :], mask_r[:, 4:8, :])
            syn.dma_start(bo_sb[:], bob_d[:])
            for h in range(2):
                for j in range(4):
                    dma_w(syn, wo_sb, wo_r, 2 * j, 2 * j + 2, h * 512, (h + 1) * 512)

            make_identity(nc, ident[:])

            # ---- projections ----
            # qT[co, t] for the 1024 queries (tl offset 8); superblocks of
            # (256, 256, 512) tokens so the first groups start on minimal DMA
            for q0, wdt in ((0, 256), (256, 256), (512, 512)):
                for cc in range(CC):
                    ps_full = ps_big.tile([P, 512], f32, tag="big")
                    ps = ps_full[:, :wdt]
                    for ci in range(CC):
                        nc.tensor.matmul(
                            ps,
                            lhsT=wq_sb[:, ci, cc * P:(cc + 1) * P],
                            rhs=xT_sb[:, ci, HALF + q0: HALF + q0 + wdt],
                            start=(ci == 0),
                            stop=(ci == CC - 1),
                        )
                    if q0 == 0 and cc < 2:
                        # first two evictions on Vector (idle then): the
                        # Scalar engine is still issuing its DMA batch and a
                        # late first eviction stalls the PE on PSUM slots
                        nc.vector.tensor_scalar(
                            qT_sb[:, cc, q0: q0 + wdt],
                            ps,
                            SCALE,
                            bq_sb[:, cc: cc + 1],
                            mybir.AluOpType.mult,
                            mybir.AluOpType.add,
                        )
                    else:
                        nc.scalar.activation(
                            qT_sb[:, cc, q0: q0 + wdt],
                            ps,
                            AF.Identity,
                            bias=bq_sb[:, cc: cc + 1],
                            scale=SCALE,
                        )
            # kT[co, tl] over the padded 1040 kv range
            for off, wdt in ((0, 512), (512, 512), (1024, 16)):
                for cc in range(CC):
                    if wdt == 512:
                        ps = ps_big.tile([P, 512], f32, tag="big", name="ps_k")
                    else:
                        ps_full = ps_sm.tile([P, P], f32, tag="sm", name="ps_kt")
                        ps = ps_full[:, :wdt]
                    for ci in range(CC):
                        nc.tensor.matmul(
                            ps,
                            lhsT=wk_sb[:, ci, cc * P:(cc + 1) * P],
                            rhs=xT_sb[:, ci, off: off + wdt],
                            start=(ci == 0),
                            stop=(ci == CC - 1),
                        )
                    nc.scalar.activation(
                        kT_sb[:, cc, off: off + wdt],
                        ps,
                        AF.Identity,
                        bias=bk_sb[:, cc: cc + 1],
                    )
            # v[tl, co] natural layout, 8 full chunks (bv is folded into ctxT
            # later)
            for ch in range(8):
                for h in range(2):
                    ps = ps_big.tile([P, 512], f32, tag="big")
                    for ci in range(CC):
                        nc.tensor.matmul(
                            ps,
                            lhsT=xT_sb[:, ci, ch * P:(ch + 1) * P],
                            rhs=wv_sb[:, ci, h * 512:(h + 1) * 512],
                            start=(ci == 0),
                            stop=(ci == CC - 1),
                        )
                    nc.vector.tensor_copy(v_sb[:, ch, h * 512:(h + 1) * 512], ps)
            # v tail (tl 1024:1040): computed transposed with a 16-wide moving
            # dim (1k rows instead of 8k), then PE-transposed into natural
            # layout.
            for cc in range(CC):
                ps_full = ps_sm.tile([P, P], f32, tag="sm", name="ps_vt")
                ps = ps_full[:, :16]
                for ci in range(CC):
                    nc.tensor.matmul(
                        ps,
                        lhsT=wv_sb[:, ci, cc * P:(cc + 1) * P],
                        rhs=xT_sb[:, ci, 1024:TK],
                        start=(ci == 0),
                        stop=(ci == CC - 1),
                    )
                nc.vector.tensor_copy(vt16[:, cc, :], ps)
            for cc in range(CC):
                pst = ps_sm.tile([P, P], f16, tag="sm", name="ps_vtt")
                nc.tensor.transpose(pst[0:16, :], vt16[:, cc, :], ident[:])
                nc.vector.tensor_copy(
                    v_sb[0:16, 8, cc * P:(cc + 1) * P], pst[0:16, :]
                )

            # ---- attention + output projection, per 128-query block ----
            # The output projection of block b is emitted between block b+1's
            # P-transposes and its PV matmuls (software pipelining): the PE
            # then has ~4us of o-proj work to hide the PSUM->SBUF copy
            # latency of the transposed P that gates PV.
            def oproj(b, ctx_blk, nh=2):
                # nh=4 (256-wide pieces) for the final block: the last y DMA
                # is then half as large, shortening the kernel tail.
                y_sb = yp.tile([P, C], f16, tag="y")
                w = C // nh
                for h in range(nh):
                    psy_full = ps_big.tile([P, 512], f32, tag="big")
                    psy = psy_full[:, :w]
                    for ci in range(CC):
                        nc.tensor.matmul(
                            psy,
                            lhsT=ctx_blk[:, ci, :],
                            rhs=wo_sb[:, ci, h * w:(h + 1) * w],
                            start=(ci == 0),
                            stop=(ci == CC - 1),
                        )
                    nc.vector.tensor_add(
                        y_sb[:, h * w:(h + 1) * w], psy, bo_sb[:, h * w:(h + 1) * w]
                    )
                    nc.sync.dma_start(
                        y_d[b * P:(b + 1) * P, h * w:(h + 1) * w],
                        y_sb[:, h * w:(h + 1) * w],
                    )

            prev = None
            for b in range(NB):
                ps = ps_s.tile([P, WJ], f32, tag="s")
                for cc in range(CC):
                    nc.tensor.matmul(
                        ps,
                        lhsT=qT_sb[:, cc, b * P:(b + 1) * P],
                        rhs=kT_sb[:, cc, b * P: b * P + WJ],
                        start=(cc == 0),
                        stop=(cc == CC - 1),
                    )
                S = work.tile([P, WJ], f32, tag="S")
                nc.vector.tensor_add(S, ps, mask_sb[:, b, :])
                negm = work.tile([P, 1], f32, tag="negm")
                nc.vector.reduce_max(negm, S, axis=AX.X, negate=True)
                P32 = work.tile([P, WJ], f32, tag="P32")
                ssum = work.tile([P, 1], f32, tag="ssum")
                nc.scalar.activation(
                    P32, S, AF.Exp, bias=negm[:, 0:1], accum_out=ssum[:, 0:1]
                )
                rr = work.tile([P, 1], f32, tag="rr")
                nc.vector.reciprocal(rr, ssum)
                P16 = work.tile([P, WJ], f16, tag="P16")
                nc.vector.tensor_scalar_mul(P16, P32, rr[:, 0:1])

                # pt0 copy on Scalar, pt1 on Vector: the two PSUM->SBUF copies
                # of the transposed P run in parallel and neither sits behind
                # the previous block's y bias-adds in the Vector queue.
                pps0 = ps_sm.tile([P, P], f16, tag="sm", name="ps_pt0")
                nc.tensor.transpose(pps0, P16[:, 0:P], ident[:])
                pt0 = ptp.tile([P, P], f16, tag="ptt")
                nc.scalar.activation(pt0, pps0, AF.Copy)
                pps1 = ps_sm.tile([P, P], f16, tag="sm", name="ps_pt1")
                nc.tensor.transpose(pps1[0:16, :], P16[:, P:WJ], ident[:])
                pt1 = ptp.tile([P, P], f16, tag="ptt")
                nc.vector.tensor_copy(pt1[0:16, :], pps1[0:16, :])

                if prev is not None:
                    oproj(*prev)

                ctx_blk = ctxp.tile([P, CC, P], f16, tag="ctxT")
                for cs in range(CC):
                    pc_full = ps_sm.tile([P, P], f32, tag="sm", name="ps_ct")
                    pc = pc_full
                    nc.tensor.matmul(
                        pc,
                        lhsT=v_sb[:, b, cs * P:(cs + 1) * P],
                        rhs=pt0[:],
                        start=True,
                        stop=False,
                    )
                    nc.tensor.matmul(
                        pc,
                        lhsT=v_sb[0:16, b + 1, cs * P:(cs + 1) * P],
                        rhs=pt1[0:16, :],
                        start=False,
                        stop=True,
                    )
                    nc.scalar.activation(
                        ctx_blk[:, cs, :], pc, AF.Identity, bias=bv_sb[:, cs: cs + 1]
                    )

                prev = (b, ctx_blk)
            oproj(*prev, nh=4)

    _split_excess_waits(nc)
    return nc


def _host_inputs(x, Wq, bq, Wk, bk, Wv, bv, Wo, bo):
    """Build per-core input maps (shared weight arrays across cores)."""
    f16 = np.float16
    wqT = np.ascontiguousarray(np.asarray(Wq, np.float32).T).astype(f16)
    wkT = np.ascontiguousarray(np.asarray(Wk, np.float32).T).astype(f16)
    wvT = np.ascontiguousarray(np.asarray(Wv, np.float32).T).astype(f16)
    woT = np.ascontiguousarray(np.asarray(Wo, np.float32).T).astype(f16)

    def tile_bias(b_, scale=1.0):
        b_ = np.asarray(b_, np.float32) * scale
        return np.ascontiguousarray(b_.reshape(CC, P).T).astype(np.float32)

    bqs = tile_bias(bq, SCALE)
    bkt = tile_bias(bk)
    bvt = tile_bias(bv)
    bob = np.ascontiguousarray(
        np.broadcast_to(np.asarray(bo, np.float32), (P, C))
    ).astype(np.float32)

    x = np.asarray(x, np.float32)
    in_maps = []
    for core in range(N_CORES):
        bidx = core // 2
        t0 = (core % 2) * TQ
        lo = t0 - HALF
        xT = np.zeros((C, TK), f16)
        s0 = max(lo, 0)
        s1 = min(lo + TK, T)
        xT[:, s0 - lo: s1 - lo] = x[bidx, s0:s1, :].T.astype(f16)

        ii = np.arange(P)[None, :, None]
        jj = np.arange(WJ)[None, None, :]
        bb = np.arange(NB)[:, None, None]
        band = (jj - ii >= 0) & (jj - ii <= 2 * HALF)
        gk = lo + bb * P + jj
        valid = band & (gk >= 0) & (gk < T)
        mask = np.where(valid, np.float32(0.0), np.float32(-1e30))
        mask = np.ascontiguousarray(np.broadcast_to(mask, (NB, P, WJ)), np.float32)

        in_maps.append(
            {
                "xT": xT,
                "wqT": wqT,
                "wkT": wkT,
                "wvT": wvT,
                "woT": woT,
                "bqs": bqs,
                "bkt": bkt,
                "bvt": bvt,
                "bob": bob,
                "mask": mask,
            }
        )
    return in_maps


def kernel(x, Wq, bq, Wk, bk, Wv, bv, Wo, bo, window):
    global _PROGRAM, LAST_EXEC_NS
    assert int(window) == 2 * HALF

    from concourse import bass_utils

    if _PROGRAM is None:
        _PROGRAM = _build_program()
    nc = _PROGRAM

    in_maps = _host_inputs(x, Wq, bq, Wk, bk, Wv, bv, Wo, bo)
    res = bass_utils.run_bass_kernel_spmd(
        nc, in_maps, core_ids=list(range(N_CORES)), trace=TRACE
    )
    LAST_EXEC_NS = res.exec_time_ns

    out = np.empty((B, T, C), np.float32)
    for core in range(N_CORES):
        bidx = core // 2
        t0 = (core % 2) * TQ
        out[bidx, t0: t0 + TQ, :] = res.results[core]["y"].astype(np.float32)
    return out
